# revision 15
# baseline (speedup 1.0000x reference)
"""Distributed Trainium2 Bass kernel for a causal single-head attention layer.

Problem shapes (hardcoded): N=4, S=T=2048, D=1024, f32 I/O.
  q = query @ Wq.T ; k = key @ Wk.T ; v = value @ Wv.T
  y = softmax(mask(q k^T / sqrt(D))) v

Sharding over 8 NeuronCores: core c -> (batch n = c//2, parity h = c%2).
Each core owns 8 interleaved 128-row query blocks (global block G = 2j+h,
j=0..7) and 8 INTERLEAVED key/value tiles (global t-tile g = 2i+h, i=0..7).
Interleaving both axes balances the causal workload exactly AND makes the
causal loop bounds identical on the two cores of a pair (SPMD-clean):
storage index u = own tiles at u=0..7 (i order), peer tiles at u=8..15,
with score columns starting at s0 = (u%8)*128 on both cores.

Exchange: each core projects K/V for its own tiles, writes them straight
into their SBUF slots (kT cols 0:1024, vA slots 0:8 -- no DRAM round trip),
and bounces them to HBM for pair-wise AllGathers.  Only the PEER half is
read back, via conditional DMAs (cond = h / 1-h from a per-core host input
register): both gather slots are read by a cond-DMA pair of which exactly
one executes; skipped DMAs still bump semaphores so dependency counting is
unchanged.  The gpsimd queue carries ONLY collective triggers (a CC
instruction occupies its queue until the serial CC stream accepts it).
The V exchange is split in two so the attention-value phase never waits.

Device compute is fp16 (TensorEngine runs 16-bit at 2x fp32 rate) with f32
PSUM accumulation; the host pre-transposes/casts inputs so no on-device
transposes are needed:
  kT[e,t_own] = wkT-weights x xkTh           (own tiles only)
  v[t_own,e]  = xvTh.T x wvT                 (own tiles only)
  qT[e,s] = (Wq/32)T-weights x qT-inputs
  ST[t,s] = kT.T @ qT per 128-wide t-tile u, cols s0(u)..1024
  EST = exp(ST) * mask  (mask data taken from the real attn_mask input)
  y[s,e], sums[s] = EST.T @ [v | 1]  (ones-column gives softmax denominator)
  out = y * (1/sums)
"""

import numpy as np

from concourse import bass, mybir, tile, bacc
from concourse.bass_utils import run_bass_kernel_spmd

P = 128
N_BATCH = 4
S = 2048   # full query length
T = 2048   # key/value length
D = 1024   # model dim
SL = 1024  # per-core query rows
TH = 1024  # per-core owned t columns
JB = SL // P   # 8 local s-blocks per core
GT = T // P    # 16 t-tiles
GH = TH // P   # 8 t-tiles owned locally
DO = D // P    # 8 outer tiles of the contraction dim
EO = D // P    # 8 outer tiles of the e dim
N_CORES = 8
PAIRS = [[0, 1], [2, 3], [4, 5], [6, 7]]

_GRAPH_CACHE = {}


def _build_graph():
    if "nc" in _GRAPH_CACHE:
        return _GRAPH_CACHE["nc"]

    fp16 = mybir.dt.float16
    f32 = mybir.dt.float32
    i32 = mybir.dt.int32

    nc = bacc.Bacc("TRN2", target_bir_lowering=False, debug=False,
                   num_devices=N_CORES)

    xqT_e = nc.dram_tensor("xqT", [D, SL], fp16, kind="ExternalInput")
    xkT_e = nc.dram_tensor("xkTh", [D, TH], fp16, kind="ExternalInput")
    xvT_e = nc.dram_tensor("xvTh", [D, TH], fp16, kind="ExternalInput")
    wqT_e = nc.dram_tensor("wqT", [D, D], fp16, kind="ExternalInput")
    wkT_e = nc.dram_tensor("wkT", [D, D], fp16, kind="ExternalInput")
    wvT_e = nc.dram_tensor("wvT", [D, D], fp16, kind="ExternalInput")
    mask_e = nc.dram_tensor("maskT", [GT, P, P], fp16, kind="ExternalInput")
    hsel_e = nc.dram_tensor("hsel", [1, 2], i32, kind="ExternalInput")
    out_e = nc.dram_tensor("out", [SL, D], fp16, kind="ExternalOutput")

    # collective bounce buffers (pair-wise AllGather works HBM->HBM).
    bk1_e = nc.dram_tensor("bk1", [P, EO, 512], fp16, kind="Internal")
    # K chunk 2 + both V halves ride ONE AllGather (fewer serial CC-stream
    # slots): layout [bk2 | bv1 | bv2], 4096 fp16 each per partition
    bkv_e = nc.dram_tensor("bkv", [P, 3 * 4096], fp16, kind="Internal")
    gk1_e = nc.dram_tensor("gk1", [2, P, EO, 512], fp16, kind="Internal")
    gkv_e = nc.dram_tensor("gkv", [2, P, 3 * 4096], fp16, kind="Internal")

    xq_r = xqT_e.ap().rearrange("(o p) s -> p o s", p=P)
    xk_r = xkT_e.ap().rearrange("(o p) t -> p o t", p=P)
    xv_r = xvT_e.ap().rearrange("(o p) t -> p o t", p=P)
    wq_r = wqT_e.ap().rearrange("(o p) e -> p o e", p=P)
    wk_r = wkT_e.ap().rearrange("(o p) e -> p o e", p=P)
    wv_r = wvT_e.ap().rearrange("(o p) e -> p o e", p=P)

    with tile.TileContext(nc) as tc:
        with tc.tile_pool(name="persist", bufs=1) as persist:
            qT = persist.tile([P, EO, SL], fp16)       # [e-part, e-outer, s]
            kT = persist.tile([P, EO, T], fp16)        # [e-part, e-outer, u*P]
            vA = persist.tile([P, GT, D + 1], fp16)    # [t-part, u, e+1]
            maskT = persist.tile([P, GT, P], fp16)     # [t-part, u, s-local]
            recip = persist.tile([P, JB], f32)
            hs = persist.tile([1, 2], i32)

            nc.vector.memset(vA[:, :, D:D + 1], 1.0)

            # per-core rank selectors for the conditional peer readbacks
            nc.sync.dma_start(hs[:], hsel_e.ap())
            _rga = nc.sync.alloc_register("r_h")     # 1 iff h==1 (peer=rank0)
            _rgb = nc.sync.alloc_register("r_nh")    # 1 iff h==0 (peer=rank1)
            nc.sync.reg_load(_rga, hs[0:1, 0:1])
            nc.sync.reg_load(_rgb, hs[0:1, 1:2])
            r_h = nc.sync.snap(_rga, donate=True, min_val=0, max_val=1)
            r_nh = nc.sync.snap(_rgb, donate=True, min_val=0, max_val=1)

            with (
                tc.tile_pool(name="weights", bufs=3) as wpool,
                tc.tile_pool(name="xin", bufs=3) as xpool,
            ):
                wk = wpool.tile([P, DO, D], fp16, tag="w", name="wk")
                wv = wpool.tile([P, DO, D], fp16, tag="w", name="wv")
                wq = wpool.tile([P, DO, D], fp16, tag="w", name="wq")

                # weights on the scalar queue, sliced per contraction tile o
                # so chunk 0's o-major matmul order streams right behind the
                # arriving slices
                for o in range(DO):
                    nc.scalar.dma_start(wk[:, o, :], wk_r[:, o, :])

                # ---- K projection (own tiles): kT[e,u*P] = wkT.T @ xkTh ----
                with tc.tile_pool(name="qpsum", bufs=8, space="PSUM") as qpsum:
                    with nc.named_scope("projK"):
                        xks = []
                        for sc in range(TH // 512):
                            xk = xpool.tile([P, DO, 512], fp16, tag="x",
                                            name=f"xk{sc}")
                            if sc == 0:
                                nc.sync.dma_start(xk[:, 0, 0:256],
                                                  xk_r[:, 0, 0:256])
                                nc.sync.dma_start(xk[:, 0, 256:512],
                                                  xk_r[:, 0, 256:512])
                                for o in range(1, DO):
                                    nc.sync.dma_start(xk[:, o, :],
                                                      xk_r[:, o, 0:512])
                            else:
                                # second chunk rides the otherwise-empty
                                # gpsimd queue (its CC doorbells are tiny
                                # and queued after)
                                nc.gpsimd.dma_start(
                                    xk[:, 0:DO // 2, :],
                                    xk_r[:, 0:DO // 2, 512:1024])
                                nc.gpsimd.dma_start(
                                    xk[:, DO // 2:DO, :],
                                    xk_r[:, DO // 2:DO, 512:1024])
                            xks.append(xk)
                        # chunk 0: o=0 warmup pass across 8 PSUM banks so
                        # the first matmuls only need the tiny o=0 slices,
                        # then m-major so the copies and the per-m bounce
                        # pieces stream out during the chunk (early K1
                        # trigger), while the per-o input slices keep the
                        # m=0 column streaming right behind the DMAs.
                        pss = [qpsum.tile([P, 512], f32, tag="qp",
                                          name=f"kp0_{m2}")
                               for m2 in range(EO)]
                        for m in range(EO):
                            nc.tensor.matmul(
                                pss[m][:, 0:256], wk[:, 0, m * P:(m + 1) * P],
                                xks[0][:, 0, 0:256],
                                start=True, stop=False, skip_group_check=True,
                            )
                        with nc.named_scope("swapK1"):
                            for m in range(EO):
                                nc.tensor.matmul(
                                    pss[m][:, 256:512],
                                    wk[:, 0, m * P:(m + 1) * P],
                                    xks[0][:, 0, 256:512],
                                    start=False, stop=False,
                                    skip_group_check=True,
                                )
                                for o in range(1, DO):
                                    nc.tensor.matmul(
                                        pss[m][:], wk[:, o, m * P:(m + 1) * P],
                                        xks[0][:, o, :],
                                        start=False, stop=(o == DO - 1),
                                        skip_group_check=True,
                                    )
                                nc.vector.tensor_copy(kT[:, m, 0:512],
                                                      pss[m][:])
                                nc.scalar.dma_start(bk1_e.ap()[:, m, :],
                                                    kT[:, m, 0:512])
                            nc.gpsimd.collective_compute(
                                "AllGather", mybir.AluOpType.bypass,
                                replica_groups=PAIRS,
                                ins=[bk1_e.ap()], outs=[gk1_e.ap()],
                            )
                        nc.scalar.dma_start(wv[:], wv_r[:])
                        nc.scalar.dma_start(wq[:], wq_r[:])
                        # chunk 1: (m, o) order -- psum slots recycle
                        for m in range(EO):
                            ps = qpsum.tile([P, 512], f32, tag="qp",
                                            name=f"kp1_{m}")
                            for o in range(DO):
                                nc.tensor.matmul(
                                    ps[:], wk[:, o, m * P:(m + 1) * P],
                                    xks[1][:, o, :],
                                    start=(o == 0), stop=(o == DO - 1),
                                )
                            nc.vector.tensor_copy(kT[:, m, 512:1024], ps[:])
                with tc.tile_pool(name="ppsum", bufs=4, space="PSUM") as ppsum:
                    # ---- V projection (own tiles): v[u*P,e] = xvTh.T @ wvT,
                    # first half now, so the V1 exchange can enter the serial
                    # CC stream BEFORE K2 (consumer order: scores need gk1
                    # first, the attention-value phase needs gv1 before gk2's
                    # last consumer) ----
                    xvs = []
                    for sc in range(TH // 512):
                        xv = xpool.tile([P, DO, 512], fp16, tag="x",
                                        name=f"xv{sc}")
                        nc.sync.dma_start(
                            xv[:], xv_r[:, :, 512 * sc:512 * (sc + 1)])
                        xvs.append(xv)

                    def proj_v_tile(m):
                        ps0 = ppsum.tile([P, 512], f32, tag="pp")
                        ps1 = ppsum.tile([P, 512], f32, tag="pp")
                        for o in range(DO):
                            lhsT = xvs[m // 4][:, o, (m % 4) * P:
                                               (m % 4 + 1) * P]
                            nc.tensor.matmul(ps0[:], lhsT, wv[:, o, 0:512],
                                             start=(o == 0),
                                             stop=(o == DO - 1))
                            nc.tensor.matmul(ps1[:], lhsT,
                                             wv[:, o, 512:1024],
                                             start=(o == 0),
                                             stop=(o == DO - 1))
                        nc.vector.tensor_copy(vA[:, m, 0:512], ps0[:])
                        nc.vector.tensor_copy(vA[:, m, 512:1024], ps1[:])

                    with nc.named_scope("projV1"):
                        for m in range(GH // 2):
                            proj_v_tile(m)
                    bkv_k2 = bkv_e.ap()[:, 0:4096].rearrange(
                        "p (o t) -> p o t", t=512)
                    bkv_v1 = bkv_e.ap()[:, 4096:8192].rearrange(
                        "p (m e) -> p m e", e=D)
                    bkv_v2 = bkv_e.ap()[:, 8192:12288].rearrange(
                        "p (m e) -> p m e", e=D)
                    nc.scalar.dma_start(bkv_k2, kT[:, :, 512:1024])
                    nc.scalar.dma_start(bkv_v1, vA[:, 0:GH // 2, 0:D])
                    with nc.named_scope("projV2"):
                        for m in range(GH // 2, GH):
                            proj_v_tile(m)
                    with nc.named_scope("swapKV"):
                        nc.scalar.dma_start(bkv_v2, vA[:, GH // 2:GH, 0:D])
                        nc.gpsimd.collective_compute(
                            "AllGather", mybir.AluOpType.bypass,
                            replica_groups=PAIRS,
                            ins=[bkv_e.ap()], outs=[gkv_e.ap()],
                        )

                    # ---- Q projection: qT[e,s] = wqT.T @ xqT ----
                    with nc.named_scope("projQ"):
                        for sc in range(SL // 512):
                            xq = xpool.tile([P, DO, 512], fp16, tag="x",
                                            name=f"xq{sc}")
                            nc.sync.dma_start(
                                xq[:], xq_r[:, :, 512 * sc:512 * (sc + 1)])
                            for m in range(EO):
                                ps = ppsum.tile([P, 512], f32, tag="pp")
                                for o in range(DO):
                                    nc.tensor.matmul(
                                        ps[:], wq[:, o, m * P:(m + 1) * P],
                                        xq[:, o, :],
                                        start=(o == 0), stop=(o == DO - 1),
                                    )
                                nc.vector.tensor_copy(
                                    qT[:, m, 512 * sc:512 * (sc + 1)], ps[:])
                        nc.scalar.dma_start(
                            maskT[:], mask_e.ap().rearrange("g p s -> p g s"))

                    # ---- conditional peer readbacks (sync queue): exactly
                    # one of each cond-pair executes; skipped DMAs still
                    # increment semaphores so downstream waits are uniform --
                    with nc.named_scope("readback"):
                        nc.sync.dma_start(kT[:, :, TH:TH + 512],
                                          gk1_e.ap()[0], cond=r_h)
                        nc.sync.dma_start(kT[:, :, TH:TH + 512],
                                          gk1_e.ap()[1], cond=r_nh)
                        for r, cnd in ((0, r_h), (1, r_nh)):
                            g = gkv_e.ap()[r]
                            nc.sync.dma_start(
                                kT[:, :, TH + 512:T],
                                g[:, 0:4096].rearrange("p (o t) -> p o t",
                                                       t=512),
                                cond=cnd)
                        for r, cnd in ((0, r_h), (1, r_nh)):
                            g = gkv_e.ap()[r]
                            nc.sync.dma_start(
                                vA[:, GH:GH + 4, 0:D],
                                g[:, 4096:8192].rearrange("p (m e) -> p m e",
                                                          e=D),
                                cond=cnd)
                        for r, cnd in ((0, r_h), (1, r_nh)):
                            g = gkv_e.ap()[r]
                            nc.sync.dma_start(
                                vA[:, GH + 4:GT, 0:D],
                                g[:, 8192:12288].rearrange("p (m e) -> p m e",
                                                           e=D),
                                cond=cnd)

            # ---- scores + exp + mask, per t-tile u (own tiles first) ----
            with tc.tile_pool(name="estp", bufs=1) as estp:
                est = estp.tile([P, GT, SL], fp16)     # [t-part, u, s]
                with (
                    nc.named_scope("scores"),
                    tc.tile_pool(name="spsum", bufs=3, space="PSUM") as spsum,
                ):
                    for u in range(GT):
                        s0 = (u % 8) * P
                        ncols = SL - s0
                        ps = spsum.tile([P, 1024], f32, tag="sp")
                        n_first = min(512, ncols)
                        for c in range(EO):
                            lhsT = kT[:, c, u * P:(u + 1) * P]
                            nc.tensor.matmul(
                                ps[:, 0:n_first], lhsT, qT[:, c, s0:s0 + n_first],
                                start=(c == 0), stop=(c == EO - 1),
                            )
                            if ncols > 512:
                                nc.tensor.matmul(
                                    ps[:, 512:ncols], lhsT, qT[:, c, s0 + 512:SL],
                                    start=(c == 0), stop=(c == EO - 1),
                                )
                        nc.scalar.activation(
                            est[:, u, s0:SL], ps[:, 0:ncols],
                            mybir.ActivationFunctionType.Exp,
                        )
                        nc.vector.tensor_mul(
                            out=est[:, u, s0:s0 + P],
                            in0=est[:, u, s0:s0 + P],
                            in1=maskT[:, u, :],
                        )

                # ---- attention-value + row sums + normalize, per block j ----
                with nc.named_scope("av"):
                    with (
                        tc.tile_pool(name="bpsum", bufs=2, space="PSUM") as bpsum,
                        tc.tile_pool(name="yout", bufs=3) as ypool,
                    ):
                        for j in range(JB):
                            us = list(range(0, j + 1)) + \
                                 list(range(GH, GH + j + 1))
                            ps = bpsum.tile([P, D + 1], f32, tag="bp")
                            for idx, u in enumerate(us):
                                lhsT = est[:, u, j * P:(j + 1) * P]
                                st = (idx == 0)
                                sp = (idx == len(us) - 1)
                                # sums column first: on the last u the
                                # reciprocal overlaps the trailing matmuls
                                nc.tensor.matmul(ps[:, 1024:1025], lhsT,
                                                 vA[:, u, 1024:1025],
                                                 start=st, stop=sp)
                                nc.tensor.matmul(ps[:, 0:512], lhsT,
                                                 vA[:, u, 0:512],
                                                 start=st, stop=sp)
                                nc.tensor.matmul(ps[:, 512:1024], lhsT,
                                                 vA[:, u, 512:1024],
                                                 start=st, stop=sp)
                            nc.vector.reciprocal(recip[:, j:j + 1],
                                                 ps[:, D:D + 1])
                            yt = ypool.tile([P, D], fp16, tag="y")
                            for q2 in range(2):
                                c0 = q2 * 512
                                nc.vector.tensor_scalar_mul(
                                    yt[:, c0:c0 + 512], ps[:, c0:c0 + 512],
                                    recip[:, j:j + 1])
                                nc.scalar.dma_start(
                                    out_e.ap()[j * P:(j + 1) * P, c0:c0 + 512],
                                    yt[:, c0:c0 + 512])

    nc.compile()
    _GRAPH_CACHE["nc"] = nc
    return nc


def _s_index(h):
    return np.concatenate([np.arange(P) + (2 * j + h) * P for j in range(JB)])


def _t_index(h):
    return np.concatenate([np.arange(P) + (2 * i + h) * P for i in range(GH)])


def _prepare_in_maps(query, key, value, attn_mask, Wq, Wk, Wv):
    query = np.asarray(query, np.float32)
    key = np.asarray(key, np.float32)
    value = np.asarray(value, np.float32)
    attn_mask = np.asarray(attn_mask)
    Wq = np.asarray(Wq, np.float32)
    Wk = np.asarray(Wk, np.float32)
    Wv = np.asarray(Wv, np.float32)

    scale = np.float32(1.0 / np.sqrt(np.float32(D)))
    wqT = np.ascontiguousarray((Wq * scale).T).astype(np.float16)  # [d, e]
    wkT = np.ascontiguousarray(Wk.T).astype(np.float16)
    wvT = np.ascontiguousarray(Wv.T).astype(np.float16)

    in_maps = []
    for c in range(N_CORES):
        n, h = c // 2, c % 2
        sidx = _s_index(h)
        tidx = _t_index(h)
        xqT = np.ascontiguousarray(query[n][sidx].T).astype(np.float16)
        kTn = key[n].T   # [d, t]
        vTn = value[n].T
        xkTh = np.ascontiguousarray(kTn[:, tidx]).astype(np.float16)
        xvTh = np.ascontiguousarray(vTn[:, tidx]).astype(np.float16)
        maskT = np.empty((GT, P, P), np.float16)
        for u in range(GT):
            j0 = u % 8
            G0 = 2 * j0 + h
            g = 2 * j0 + (h if u < GH else 1 - h)
            blk = attn_mask[G0 * P:(G0 + 1) * P, g * P:(g + 1) * P]  # [s, t]
            maskT[u] = np.ascontiguousarray(blk.T).astype(np.float16)
        hsel = np.array([[h, 1 - h]], np.int32)
        in_maps.append({
            "xqT": xqT, "xkTh": xkTh, "xvTh": xvTh,
            "wqT": wqT, "wkT": wkT, "wvT": wvT, "maskT": maskT,
            "hsel": hsel,
        })
    return in_maps


def run(trace=False, **inputs):
    nc = _build_graph()
    in_maps = _prepare_in_maps(**inputs)
    res = run_bass_kernel_spmd(nc, in_maps, list(range(N_CORES)), trace=trace)
    out = np.empty((N_BATCH, S, D), np.float32)
    for c in range(N_CORES):
        n, h = c // 2, c % 2
        out[n][_s_index(h)] = res.results[c]["out"].astype(np.float32)
    return out, res


def kernel(**inputs):
    out, _ = run(trace=False, **inputs)
    return out


# revision 19
# speedup vs baseline: 1.2567x; 1.2567x over previous
"""Distributed Trainium2 Bass kernel for a causal single-head attention layer.

Problem shapes (hardcoded): N=4, S=T=2048, D=1024, f32 I/O.
  q = query @ Wq.T ; k = key @ Wk.T ; v = value @ Wv.T
  y = softmax(mask(q k^T / sqrt(D))) v

Sharding over 8 NeuronCores: core c -> (batch n = c//2, parity h = c%2).
Each core owns 8 interleaved 128-row query blocks (global block G = 2j+h,
j=0..7) and 8 INTERLEAVED key/value tiles (global t-tile g = 2i+h, i=0..7).
Interleaving both axes balances the causal workload exactly AND makes the
causal loop bounds identical on the two cores of a pair (SPMD-clean):
storage index u = own tiles at u=0..7 (i order), peer tiles at u=8..15,
with score columns starting at s0 = (u%8)*128 on both cores.

Exchange: each core projects K/V for its own tiles, writes them straight
into their SBUF slots (kT cols 0:1024, vA slots 0:8 -- no DRAM round trip),
and bounces them to HBM for pair-wise AllGathers.  Only the PEER half is
read back, via conditional DMAs (cond = h / 1-h from a per-core host input
register): both gather slots are read by a cond-DMA pair of which exactly
one executes; skipped DMAs still bump semaphores so dependency counting is
unchanged.  The gpsimd queue carries ONLY collective triggers (a CC
instruction occupies its queue until the serial CC stream accepts it).
The V exchange is split in two so the attention-value phase never waits.

Device compute is fp16 (TensorEngine runs 16-bit at 2x fp32 rate) with f32
PSUM accumulation; the host pre-transposes/casts inputs so no on-device
transposes are needed:
  kT[e,t_own] = wkT-weights x xkTh           (own tiles only)
  v[t_own,e]  = xvTh.T x wvT                 (own tiles only)
  qT[e,s] = (Wq/32)T-weights x qT-inputs
  ST[t,s] = kT.T @ qT per 128-wide t-tile u, cols s0(u)..1024
  EST = exp(ST) * mask  (mask data taken from the real attn_mask input)
  y[s,e], sums[s] = EST.T @ [v | 1]  (ones-column gives softmax denominator)
  out = y * (1/sums)
"""

import numpy as np

from concourse import bass, mybir, tile, bacc
from concourse.bass_utils import run_bass_kernel_spmd

P = 128
N_BATCH = 4
S = 2048   # full query length
T = 2048   # key/value length
D = 1024   # model dim
SL = 1024  # per-core query rows
TH = 1024  # per-core owned t columns
JB = SL // P   # 8 local s-blocks per core
GT = T // P    # 16 t-tiles
GH = TH // P   # 8 t-tiles owned locally
DO = D // P    # 8 outer tiles of the contraction dim
EO = D // P    # 8 outer tiles of the e dim
N_CORES = 8
PAIRS = [[0, 1], [2, 3], [4, 5], [6, 7]]

_GRAPH_CACHE = {}


def _build_graph():
    if "nc" in _GRAPH_CACHE:
        return _GRAPH_CACHE["nc"]

    fp16 = mybir.dt.float16
    f32 = mybir.dt.float32
    i32 = mybir.dt.int32

    nc = bacc.Bacc("TRN2", target_bir_lowering=False, debug=False,
                   num_devices=N_CORES)

    xqT_e = nc.dram_tensor("xqT", [D, SL], fp16, kind="ExternalInput")
    xkT_e = nc.dram_tensor("xkTh", [D, TH], fp16, kind="ExternalInput")
    xvT_e = nc.dram_tensor("xvTh", [D, TH], fp16, kind="ExternalInput")
    wqT_e = nc.dram_tensor("wqT", [D, D], fp16, kind="ExternalInput")
    wkT_e = nc.dram_tensor("wkT", [D, D], fp16, kind="ExternalInput")
    wvT_e = nc.dram_tensor("wvT", [D, D], fp16, kind="ExternalInput")
    mask_e = nc.dram_tensor("maskT", [GT, P, P], fp16, kind="ExternalInput")
    hsel_e = nc.dram_tensor("hsel", [1, 2], i32, kind="ExternalInput")
    out_e = nc.dram_tensor("out", [SL, D], fp16, kind="ExternalOutput")

    # collective bounce buffers (pair-wise AllGather works HBM->HBM).
    bk1_e = nc.dram_tensor("bk1", [P, EO, 512], fp16, kind="Internal")
    bk2_e = nc.dram_tensor("bk2", [P, EO, 512], fp16, kind="Internal")
    bv1_e = nc.dram_tensor("bv1", [P, GH // 2, D], fp16, kind="Internal")
    bv2_e = nc.dram_tensor("bv2", [P, GH // 2, D], fp16, kind="Internal")
    gk1_e = nc.dram_tensor("gk1", [2, P, EO, 512], fp16, kind="Internal")
    gk2_e = nc.dram_tensor("gk2", [2, P, EO, 512], fp16, kind="Internal")
    gv1_e = nc.dram_tensor("gv1", [2, P, GH // 2, D], fp16, kind="Internal")
    gv2_e = nc.dram_tensor("gv2", [2, P, GH // 2, D], fp16, kind="Internal")

    xq_r = xqT_e.ap().rearrange("(o p) s -> p o s", p=P)
    xk_r = xkT_e.ap().rearrange("(o p) t -> p o t", p=P)
    xv_r = xvT_e.ap().rearrange("(o p) t -> p o t", p=P)
    wq_r = wqT_e.ap().rearrange("(o p) e -> p o e", p=P)
    wk_r = wkT_e.ap().rearrange("(o p) e -> p o e", p=P)
    wv_r = wvT_e.ap().rearrange("(o p) e -> p o e", p=P)

    with tile.TileContext(nc) as tc:
        with tc.tile_pool(name="persist", bufs=1) as persist:
            qT = persist.tile([P, EO, SL], fp16)       # [e-part, e-outer, s]
            kT = persist.tile([P, EO, T], fp16)        # [e-part, e-outer, u*P]
            vA = persist.tile([P, GT, D + 1], fp16)    # [t-part, u, e+1]
            maskT = persist.tile([P, GT, P], fp16)     # [t-part, u, s-local]
            recip = persist.tile([P, JB], f32)
            hs = persist.tile([1, 2], i32)

            nc.vector.memset(vA[:, :, D:D + 1], 1.0)

            # per-core rank selectors for the conditional peer readbacks
            nc.sync.dma_start(hs[:], hsel_e.ap())
            _rga = nc.sync.alloc_register("r_h")     # 1 iff h==1 (peer=rank0)
            _rgb = nc.sync.alloc_register("r_nh")    # 1 iff h==0 (peer=rank1)
            nc.sync.reg_load(_rga, hs[0:1, 0:1])
            nc.sync.reg_load(_rgb, hs[0:1, 1:2])
            r_h = nc.sync.snap(_rga, donate=True, min_val=0, max_val=1)
            r_nh = nc.sync.snap(_rgb, donate=True, min_val=0, max_val=1)

            with (
                tc.tile_pool(name="weights", bufs=3) as wpool,
                tc.tile_pool(name="xin", bufs=3) as xpool,
            ):
                wk = wpool.tile([P, DO, D], fp16, tag="w", name="wk")
                wv = wpool.tile([P, DO, D], fp16, tag="w", name="wv")
                wq = wpool.tile([P, DO, D], fp16, tag="w", name="wq")

                # weights on the scalar queue, sliced per contraction tile o
                # so chunk 0's o-major matmul order streams right behind the
                # arriving slices
                for o in range(DO):
                    nc.scalar.dma_start(wk[:, o, :], wk_r[:, o, :])

                # ---- K projection (own tiles): kT[e,u*P] = wkT.T @ xkTh ----
                with tc.tile_pool(name="qpsum", bufs=8, space="PSUM") as qpsum:
                    with nc.named_scope("projK"):
                        xks = []
                        for sc in range(TH // 512):
                            xk = xpool.tile([P, DO, 512], fp16, tag="x",
                                            name=f"xk{sc}")
                            if sc == 0:
                                nc.sync.dma_start(xk[:, 0, 0:256],
                                                  xk_r[:, 0, 0:256])
                                nc.sync.dma_start(xk[:, 0, 256:512],
                                                  xk_r[:, 0, 256:512])
                                for o in range(1, DO):
                                    nc.sync.dma_start(xk[:, o, :],
                                                      xk_r[:, o, 0:512])
                            else:
                                nc.sync.dma_start(
                                    xk[:, 0:DO // 2, :],
                                    xk_r[:, 0:DO // 2, 512:1024])
                                nc.sync.dma_start(
                                    xk[:, DO // 2:DO, :],
                                    xk_r[:, DO // 2:DO, 512:1024])
                            xks.append(xk)
                        # chunk 0: o=0 warmup pass across 8 PSUM banks so
                        # the first matmuls only need the tiny o=0 slices,
                        # then m-major so the copies and the per-m bounce
                        # pieces stream out during the chunk (early K1
                        # trigger), while the per-o input slices keep the
                        # m=0 column streaming right behind the DMAs.
                        pss = [qpsum.tile([P, 512], f32, tag="qp",
                                          name=f"kp0_{m2}")
                               for m2 in range(EO)]
                        for m in range(EO):
                            nc.tensor.matmul(
                                pss[m][:, 0:256], wk[:, 0, m * P:(m + 1) * P],
                                xks[0][:, 0, 0:256],
                                start=True, stop=False, skip_group_check=True,
                            )
                        with nc.named_scope("swapK1"):
                            for m in range(EO):
                                nc.tensor.matmul(
                                    pss[m][:, 256:512],
                                    wk[:, 0, m * P:(m + 1) * P],
                                    xks[0][:, 0, 256:512],
                                    start=False, stop=False,
                                    skip_group_check=True,
                                )
                                for o in range(1, DO):
                                    nc.tensor.matmul(
                                        pss[m][:], wk[:, o, m * P:(m + 1) * P],
                                        xks[0][:, o, :],
                                        start=False, stop=(o == DO - 1),
                                        skip_group_check=True,
                                    )
                                nc.vector.tensor_copy(kT[:, m, 0:512],
                                                      pss[m][:])
                                nc.scalar.dma_start(bk1_e.ap()[:, m, :],
                                                    kT[:, m, 0:512])
                            nc.gpsimd.collective_compute(
                                "AllGather", mybir.AluOpType.bypass,
                                replica_groups=PAIRS,
                                ins=[bk1_e.ap()], outs=[gk1_e.ap()],
                            )
                        nc.scalar.dma_start(wv[:], wv_r[:])
                        nc.scalar.dma_start(wq[:], wq_r[:])
                        # chunk 1: (m, o) order -- psum slots recycle
                        for m in range(EO):
                            ps = qpsum.tile([P, 512], f32, tag="qp",
                                            name=f"kp1_{m}")
                            for o in range(DO):
                                nc.tensor.matmul(
                                    ps[:], wk[:, o, m * P:(m + 1) * P],
                                    xks[1][:, o, :],
                                    start=(o == 0), stop=(o == DO - 1),
                                )
                            nc.vector.tensor_copy(kT[:, m, 512:1024], ps[:])
                with tc.tile_pool(name="ppsum", bufs=4, space="PSUM") as ppsum:
                    # ---- V projection (own tiles): v[u*P,e] = xvTh.T @ wvT,
                    # first half now, so the V1 exchange can enter the serial
                    # CC stream BEFORE K2 (consumer order: scores need gk1
                    # first, the attention-value phase needs gv1 before gk2's
                    # last consumer) ----
                    xvs = []
                    for sc in range(TH // 512):
                        xv = xpool.tile([P, DO, 512], fp16, tag="x",
                                        name=f"xv{sc}")
                        nc.sync.dma_start(
                            xv[:], xv_r[:, :, 512 * sc:512 * (sc + 1)])
                        xvs.append(xv)

                    def proj_v_tile(m):
                        ps0 = ppsum.tile([P, 512], f32, tag="pp")
                        ps1 = ppsum.tile([P, 512], f32, tag="pp")
                        for o in range(DO):
                            lhsT = xvs[m // 4][:, o, (m % 4) * P:
                                               (m % 4 + 1) * P]
                            nc.tensor.matmul(ps0[:], lhsT, wv[:, o, 0:512],
                                             start=(o == 0),
                                             stop=(o == DO - 1))
                            nc.tensor.matmul(ps1[:], lhsT,
                                             wv[:, o, 512:1024],
                                             start=(o == 0),
                                             stop=(o == DO - 1))
                        nc.vector.tensor_copy(vA[:, m, 0:512], ps0[:])
                        nc.vector.tensor_copy(vA[:, m, 512:1024], ps1[:])

                    with nc.named_scope("projV1"):
                        for m in range(GH // 2):
                            proj_v_tile(m)
                    with nc.named_scope("swapV1"):
                        nc.scalar.dma_start(bv1_e.ap(), vA[:, 0:GH // 2, 0:D])
                        nc.gpsimd.collective_compute(
                            "AllGather", mybir.AluOpType.bypass,
                            replica_groups=PAIRS,
                            ins=[bv1_e.ap()], outs=[gv1_e.ap()],
                        )
                    # second K chunk exchange enters the stream after V1
                    with nc.named_scope("swapK2"):
                        nc.scalar.dma_start(bk2_e.ap(), kT[:, :, 512:1024])
                        nc.gpsimd.collective_compute(
                            "AllGather", mybir.AluOpType.bypass,
                            replica_groups=PAIRS,
                            ins=[bk2_e.ap()], outs=[gk2_e.ap()],
                        )
                    with nc.named_scope("projV2"):
                        for m in range(GH // 2, GH):
                            proj_v_tile(m)
                    with nc.named_scope("swapV2"):
                        nc.scalar.dma_start(
                            bv2_e.ap(), vA[:, GH // 2:GH, 0:D])
                        nc.gpsimd.collective_compute(
                            "AllGather", mybir.AluOpType.bypass,
                            replica_groups=PAIRS,
                            ins=[bv2_e.ap()], outs=[gv2_e.ap()],
                        )

                    # ---- Q projection: qT[e,s] = wqT.T @ xqT ----
                    with nc.named_scope("projQ"):
                        for sc in range(SL // 512):
                            xq = xpool.tile([P, DO, 512], fp16, tag="x",
                                            name=f"xq{sc}")
                            nc.sync.dma_start(
                                xq[:], xq_r[:, :, 512 * sc:512 * (sc + 1)])
                            for m in range(EO):
                                ps = ppsum.tile([P, 512], f32, tag="pp")
                                for o in range(DO):
                                    nc.tensor.matmul(
                                        ps[:], wq[:, o, m * P:(m + 1) * P],
                                        xq[:, o, :],
                                        start=(o == 0), stop=(o == DO - 1),
                                    )
                                nc.vector.tensor_copy(
                                    qT[:, m, 512 * sc:512 * (sc + 1)], ps[:])
                        nc.scalar.dma_start(
                            maskT[:], mask_e.ap().rearrange("g p s -> p g s"))

                    # ---- conditional peer readbacks (sync queue): exactly
                    # one of each cond-pair executes; skipped DMAs still
                    # increment semaphores so downstream waits are uniform --
                    with nc.named_scope("readback"):
                        nc.sync.dma_start(kT[:, :, TH:TH + 512],
                                          gk1_e.ap()[0], cond=r_h)
                        nc.sync.dma_start(kT[:, :, TH:TH + 512],
                                          gk1_e.ap()[1], cond=r_nh)
                        nc.sync.dma_start(vA[:, GH:GH + 4, 0:D],
                                          gv1_e.ap()[0], cond=r_h)
                        nc.sync.dma_start(vA[:, GH:GH + 4, 0:D],
                                          gv1_e.ap()[1], cond=r_nh)
                        nc.sync.dma_start(kT[:, :, TH + 512:T],
                                          gk2_e.ap()[0], cond=r_h)
                        nc.sync.dma_start(kT[:, :, TH + 512:T],
                                          gk2_e.ap()[1], cond=r_nh)
                        nc.sync.dma_start(vA[:, GH + 4:GT, 0:D],
                                          gv2_e.ap()[0], cond=r_h)
                        nc.sync.dma_start(vA[:, GH + 4:GT, 0:D],
                                          gv2_e.ap()[1], cond=r_nh)

            # ---- scores + exp + mask, per t-tile u (own tiles first) ----
            with tc.tile_pool(name="estp", bufs=1) as estp:
                est = estp.tile([P, GT, SL], fp16)     # [t-part, u, s]
                with (
                    nc.named_scope("scores"),
                    tc.tile_pool(name="spsum", bufs=3, space="PSUM") as spsum,
                ):
                    for u in range(GT):
                        s0 = (u % 8) * P
                        ncols = SL - s0
                        ps = spsum.tile([P, 1024], f32, tag="sp")
                        n_first = min(512, ncols)
                        for c in range(EO):
                            lhsT = kT[:, c, u * P:(u + 1) * P]
                            nc.tensor.matmul(
                                ps[:, 0:n_first], lhsT, qT[:, c, s0:s0 + n_first],
                                start=(c == 0), stop=(c == EO - 1),
                            )
                            if ncols > 512:
                                nc.tensor.matmul(
                                    ps[:, 512:ncols], lhsT, qT[:, c, s0 + 512:SL],
                                    start=(c == 0), stop=(c == EO - 1),
                                )
                        nc.scalar.activation(
                            est[:, u, s0:SL], ps[:, 0:ncols],
                            mybir.ActivationFunctionType.Exp,
                        )
                        nc.vector.tensor_mul(
                            out=est[:, u, s0:s0 + P],
                            in0=est[:, u, s0:s0 + P],
                            in1=maskT[:, u, :],
                        )

                # ---- attention-value + row sums + normalize, per block j ----
                with nc.named_scope("av"):
                    with (
                        tc.tile_pool(name="bpsum", bufs=2, space="PSUM") as bpsum,
                        tc.tile_pool(name="yout", bufs=3) as ypool,
                    ):
                        for j in range(JB):
                            us = list(range(0, j + 1)) + \
                                 list(range(GH, GH + j + 1))
                            ps = bpsum.tile([P, D + 1], f32, tag="bp")
                            for idx, u in enumerate(us):
                                lhsT = est[:, u, j * P:(j + 1) * P]
                                st = (idx == 0)
                                sp = (idx == len(us) - 1)
                                # sums column first: on the last u the
                                # reciprocal overlaps the trailing matmuls
                                nc.tensor.matmul(ps[:, 1024:1025], lhsT,
                                                 vA[:, u, 1024:1025],
                                                 start=st, stop=sp)
                                nc.tensor.matmul(ps[:, 0:512], lhsT,
                                                 vA[:, u, 0:512],
                                                 start=st, stop=sp)
                                nc.tensor.matmul(ps[:, 512:1024], lhsT,
                                                 vA[:, u, 512:1024],
                                                 start=st, stop=sp)
                            nc.vector.reciprocal(recip[:, j:j + 1],
                                                 ps[:, D:D + 1])
                            yt = ypool.tile([P, D], fp16, tag="y")
                            for q2 in range(2):
                                c0 = q2 * 512
                                nc.vector.tensor_scalar_mul(
                                    yt[:, c0:c0 + 512], ps[:, c0:c0 + 512],
                                    recip[:, j:j + 1])
                                nc.scalar.dma_start(
                                    out_e.ap()[j * P:(j + 1) * P, c0:c0 + 512],
                                    yt[:, c0:c0 + 512])

    nc.compile()
    _GRAPH_CACHE["nc"] = nc
    return nc


def _s_index(h):
    return np.concatenate([np.arange(P) + (2 * j + h) * P for j in range(JB)])


def _t_index(h):
    return np.concatenate([np.arange(P) + (2 * i + h) * P for i in range(GH)])


def _prepare_in_maps(query, key, value, attn_mask, Wq, Wk, Wv):
    query = np.asarray(query, np.float32)
    key = np.asarray(key, np.float32)
    value = np.asarray(value, np.float32)
    attn_mask = np.asarray(attn_mask)
    Wq = np.asarray(Wq, np.float32)
    Wk = np.asarray(Wk, np.float32)
    Wv = np.asarray(Wv, np.float32)

    scale = np.float32(1.0 / np.sqrt(np.float32(D)))
    wqT = np.ascontiguousarray((Wq * scale).T).astype(np.float16)  # [d, e]
    wkT = np.ascontiguousarray(Wk.T).astype(np.float16)
    wvT = np.ascontiguousarray(Wv.T).astype(np.float16)

    in_maps = []
    for c in range(N_CORES):
        n, h = c // 2, c % 2
        sidx = _s_index(h)
        tidx = _t_index(h)
        xqT = np.ascontiguousarray(query[n][sidx].T).astype(np.float16)
        kTn = key[n].T   # [d, t]
        vTn = value[n].T
        xkTh = np.ascontiguousarray(kTn[:, tidx]).astype(np.float16)
        xvTh = np.ascontiguousarray(vTn[:, tidx]).astype(np.float16)
        maskT = np.empty((GT, P, P), np.float16)
        for u in range(GT):
            j0 = u % 8
            G0 = 2 * j0 + h
            g = 2 * j0 + (h if u < GH else 1 - h)
            blk = attn_mask[G0 * P:(G0 + 1) * P, g * P:(g + 1) * P]  # [s, t]
            maskT[u] = np.ascontiguousarray(blk.T).astype(np.float16)
        hsel = np.array([[h, 1 - h]], np.int32)
        in_maps.append({
            "xqT": xqT, "xkTh": xkTh, "xvTh": xvTh,
            "wqT": wqT, "wkT": wkT, "wvT": wvT, "maskT": maskT,
            "hsel": hsel,
        })
    return in_maps


def run(trace=False, **inputs):
    nc = _build_graph()
    in_maps = _prepare_in_maps(**inputs)
    res = run_bass_kernel_spmd(nc, in_maps, list(range(N_CORES)), trace=trace)
    out = np.empty((N_BATCH, S, D), np.float32)
    for c in range(N_CORES):
        n, h = c // 2, c % 2
        out[n][_s_index(h)] = res.results[c]["out"].astype(np.float32)
    return out, res


def kernel(**inputs):
    out, _ = run(trace=False, **inputs)
    return out
